# revision 13
# baseline (speedup 1.0000x reference)
"""Sparse multi-head attention (B=4, S=2048, F=512, H=8, D=64) on 8 trn2 cores.

Sharding: core c handles batch b = c % 4 and heads [hg*4, hg*4+4) with
hg = c // 4.  Zero duplicated FLOPs.

Device-side layout ("scores transposed"), heads processed in PAIRS so the
q/k projection runs M=128 matmuls (head j0 in partitions 0:64, j1 in 64:128):
  qtP/ktP [d, s] = Wqk_pair^T @ X_b^T   (fp16 matmul; f32 psum + bias -> DVE)
  V [t, d]  computed DIRECTLY via matmul(lhsT=X^T chunk [f,t], rhs=Wv [f,d4])
      (no PE transposes; bias added on Pool engine; ones col -> denominator)
  S^T [t, s]  = matmul(lhsT=Kt [d, t128], rhs=Qt [d, s512])  (fp16, f32 acc)
  E = exp(S^T): 3/4 of t-blocks on ACT (bf16 out, no max-subtraction:
      |scores| <= ~18); 1/4 via DVE fastexp bit-trick
      (int32 <- s*2^23*log2e + magic; bitcast f32) + Pool mask-multiply,
      to keep ACT below the PE critical path.
  E = E * maskT (bf16 multiplicative mask; DVE 4x mode on ACT path)
  Y^T_aug [65, s] += matmul(lhsT=V_aug [t128, 65], rhs=E)   (bf16)
      where V_aug has a ones column => row 64 = sum of masked exp
Host divides by the denominator and interleaves heads into the output.
The 1/sqrt(H) score scale is folded into Wq/bq on the host.
Pair-1 q/k projection matmuls are interleaved into pair-0's last attention
pass; V projection is interleaved into the first pass.
"""

import sys

for _p in ("/opt/trn_rl_repo", "/root/.axon_site/_ro/trn_rl_repo"):
    if _p not in sys.path:
        sys.path.insert(0, _p)

from contextlib import ExitStack

import ml_dtypes
import numpy as np

import concourse.bacc as bacc
import concourse.tile as tile
from concourse import bass_utils, mybir

B, S, F, H, D = 4, 2048, 512, 8, 64
HPC = H // 2  # heads per core (4): 2 head-groups x 4 batches = 8 cores
NPAIR = HPC // 2  # head pairs per core (2)
N_CORES = 8
NF = F // 128  # 4 f-chunks of 128
NT = S // 128  # 16 t-blocks
NSP = S // 1024  # 2 query-block pairs of 1024

F32 = mybir.dt.float32
I32 = mybir.dt.int32
I16 = mybir.dt.int16
BF16 = mybir.dt.bfloat16
FP16 = mybir.dt.float16
AF = mybir.ActivationFunctionType
ALU = mybir.AluOpType

# fastexp: exp(s) ~= bitcast_bf16(int16(s * 128*log2(e) + (127*128 - 6.9)))
# (bf16 bit trick; magic tuned offline to minimize final attention error)
FE_C1 = 184.66497
FE_C2 = 16249.1
FAST_TB = 2  # tb % 4 == FAST_TB uses the DVE/Pool fastexp path (None = off)


def build_nc():
    nc = bacc.Bacc(
        "TRN2", target_bir_lowering=False, debug=False, num_devices=N_CORES
    )
    xt_d = nc.dram_tensor("xt", [F, S], FP16, kind="ExternalInput").ap()
    mk_d = nc.dram_tensor("msk", [S, S], BF16, kind="ExternalInput").ap()
    # per pair m: [q_j0|q_j1|k_j0|k_j1] (256 cols)
    wqk_d = nc.dram_tensor("wqk", [F, NPAIR * 256], FP16, kind="ExternalInput").ap()
    # v cols ordered [m, half, d] (256 cols)
    wv_d = nc.dram_tensor("wv", [F, NPAIR * 128], FP16, kind="ExternalInput").ap()
    bias_d = nc.dram_tensor("bias", [128, 2 * NPAIR], F32, kind="ExternalInput").ap()
    vbias_d = nc.dram_tensor("vbias", [128, NPAIR * 128], F32, kind="ExternalInput").ap()
    yt_d = nc.dram_tensor("yt", [HPC, 65, S], F32, kind="ExternalOutput").ap()
    junk_d = nc.dram_tensor("junk", [64, 512], F32)  # warmup sink (Internal)

    with ExitStack() as ctx:
        tc = ctx.enter_context(tile.TileContext(nc))
        const = ctx.enter_context(tc.tile_pool(name="const", bufs=1))

        wqk_sb = const.tile([128, NF, NPAIR * 256], FP16)
        bias_sb = const.tile([128, 2 * NPAIR], F32)
        vbias_sb = const.tile([128, NPAIR, 2, 64], F32)
        xt_sb = const.tile([128, NF, S], FP16)
        wv_sb = const.tile([128, NF, NPAIR * 128], FP16)
        mk_sb = const.tile([128, NT, S], BF16)
        # V for all heads: [t, pair, tb, half, 65]; col 64 = ones (denominator)
        v2 = const.tile([128, NPAIR, NT, 2, 65], BF16)

        # --- input DMAs, ordered for earliest compute start
        nc.sync.dma_start(bias_sb[:], bias_d)
        nc.sync.dma_start(
            vbias_sb[:], vbias_d.rearrange("p (m h d) -> p m h d", m=NPAIR, h=2)
        )
        wqk_r = wqk_d.rearrange("(c p) n -> p c n", p=128)
        for c in range(NF):
            nc.sync.dma_start(wqk_sb[:, c, :], wqk_r[:, c, :])
        xt_r = xt_d.rearrange("(c p) s -> p c s", p=128)
        for c in range(NF):
            nc.sync.dma_start(xt_sb[:, c, 0:1024], xt_r[:, c, 0:1024])
        nc.sync.dma_start(wv_sb[:], wv_d.rearrange("(c p) n -> p c n", p=128))
        mk_r = mk_d.rearrange("(t p) s -> p t s", p=128)
        nc.sync.dma_start(mk_sb[:, 0:2, 0:1024], mk_r[:, 0:2, 0:1024])
        for c in range(NF):
            nc.sync.dma_start(xt_sb[:, c, 1024:2048], xt_r[:, c, 1024:2048])
        for i in range(1, 8):
            nc.sync.dma_start(
                mk_sb[:, 2 * i : 2 * i + 2, 0:1024], mk_r[:, 2 * i : 2 * i + 2, 0:1024]
            )
        for i in range(8):
            nc.sync.dma_start(
                mk_sb[:, 2 * i : 2 * i + 2, 1024:2048],
                mk_r[:, 2 * i : 2 * i + 2, 1024:2048],
            )

        # ones columns of V_aug (col 64 of each [.., 65] block)
        nc.vector.memset(v2[:, :, :, :, 64:65], 1.0)

        qk_pool = ctx.enter_context(tc.tile_pool(name="qk", bufs=2))
        e_pool = ctx.enter_context(tc.tile_pool(name="e", bufs=6))
        ex_pool = ctx.enter_context(tc.tile_pool(name="ex", bufs=3))
        e16_pool = ctx.enter_context(tc.tile_pool(name="e16", bufs=2))
        y_pool = ctx.enter_context(tc.tile_pool(name="y", bufs=6))
        misc_ps = ctx.enter_context(tc.tile_pool(name="mps", bufs=1, space="PSUM"))
        sc_ps = ctx.enter_context(tc.tile_pool(name="sps", bufs=2, space="PSUM"))
        y_ps = ctx.enter_context(tc.tile_pool(name="yps", bufs=3, space="PSUM"))

        # --- PE warmup: junk matmuls with a full 128x128 stationary and no
        # DMA dependency, so the clock ramps and the input DMA wait is
        # covered before the real work arrives.
        NWU = 28
        wu = const.tile([128, 512], BF16)
        nc.vector.memset(wu[:], 0.0)
        pw = sc_ps.tile([128, 512], F32, tag="s", name="pw")
        for i in range(NWU):
            nc.tensor.matmul(
                pw[:], wu[:, 0:128], wu[:], start=(i == 0), stop=(i == NWU - 1)
            )
        wu_out = const.tile([64, 512], F32)
        nc.vector.tensor_copy(wu_out[:], pw[0:64, :])
        nc.sync.dma_start(junk_d.ap(), wu_out[:])

        # per-pair q/k tiles (rows 0:64 = head j0, 64:128 = head j1)
        qt_tiles = {}
        kt_tiles = {}

        def emit_qk_group(m, kind, sq):
            """4 accumulation matmuls + bias for one (pair, q|k, s-chunk)."""
            out_sb = qt_tiles[m] if kind == 0 else kt_tiles[m]
            wsl = slice(m * 256 + kind * 128, m * 256 + (kind + 1) * 128)
            bcol = 2 * m + kind
            ssl = slice(sq * 512, (sq + 1) * 512)
            pp = misc_ps.tile([128, 512], F32, tag="m", name="pp")
            for c in range(NF):
                nc.tensor.matmul(
                    pp[:],
                    wqk_sb[:, c, wsl],
                    xt_sb[:, c, ssl],
                    start=(c == 0),
                    stop=(c == NF - 1),
                )
            nc.vector.tensor_scalar(
                out_sb[:, ssl], pp[:], bias_sb[:, bcol : bcol + 1], None, op0=ALU.add
            )

        def emit_vproj(tb):
            """V[t, d] for all 4 heads of this t-block + bias on Pool."""
            tsl = slice(tb * 128, (tb + 1) * 128)
            pv = misc_ps.tile([128, NPAIR, 2, 64], F32, tag="m", name="pv")
            for c in range(NF):
                nc.tensor.matmul(
                    pv[:],
                    xt_sb[:, c, tsl],
                    wv_sb[:, c, :],
                    start=(c == 0),
                    stop=(c == NF - 1),
                )
            for m in range(NPAIR):
                # GPSIMD cannot read PSUM on trn2 -> DVE
                nc.vector.tensor_tensor(
                    v2[:, m, tb, :, 0:64], pv[:, m], vbias_sb[:, m], op=ALU.add
                )

        for m in range(NPAIR):
            qt_tiles[m] = qk_pool.tile([128, S], FP16, tag="qt", name=f"qt{m}")
            kt_tiles[m] = qk_pool.tile([128, S], FP16, tag="kt", name=f"kt{m}")

        # pair 0: k for s 0:1024 (scores tb 0..7), q for sp0
        for sq in (0, 1):
            emit_qk_group(0, 1, sq)
        for sq in (0, 1):
            emit_qk_group(0, 0, sq)

        # work queue interleaved into attention passes, keyed by
        # (pass_key, group_index, position 0|1)
        interleave = {}

        def sched(key, gi, pos, fn, *args):
            interleave.setdefault((key, gi, pos), []).append((fn, args))

        # first pass (m0, h0, sp0): k sq2/sq3, q sq2/sq3 (vproj is inlined)
        sched((0, 0, 0), 0, 1, emit_qk_group, 0, 1, 2)
        sched((0, 0, 0), 1, 0, emit_qk_group, 0, 1, 3)
        sched((0, 0, 0), 2, 0, emit_qk_group, 0, 0, 2)
        sched((0, 0, 0), 3, 0, emit_qk_group, 0, 0, 3)
        # pair-1 projection interleaved into pass (m0, h1, sp1)
        for g, (kind, sq) in enumerate(
            [(1, 0), (1, 1), (1, 2), (1, 3), (0, 0), (0, 1), (0, 2), (0, 3)]
        ):
            sched((0, 1, 1), g // 2, g % 2, emit_qk_group, 1, kind, sq)

        for m in range(NPAIR):
            qtP = qt_tiles[m]
            ktP = kt_tiles[m]
            for half in range(2):
                j = 2 * m + half
                rsl = slice(64 * half, 64 * (half + 1))
                for sp in range(NSP):
                    key = (m, half, sp)
                    e_tiles = {}

                    def emit_scores_e(tb, fast):
                        ktb = ktP[rsl, tb * 128 : (tb + 1) * 128]
                        ps = sc_ps.tile([128, 1024], F32, tag="s", name="ps")
                        nc.tensor.matmul(
                            ps[:, 0:512],
                            ktb,
                            qtP[rsl, sp * 1024 : sp * 1024 + 512],
                            start=True,
                            stop=True,
                        )
                        nc.tensor.matmul(
                            ps[:, 512:1024],
                            ktb,
                            qtP[rsl, sp * 1024 + 512 : (sp + 1) * 1024],
                            start=True,
                            stop=True,
                        )
                        e = e_pool.tile([128, 1024], BF16, tag="e", name="e")
                        e_tiles[tb] = e
                        if fast:
                            # fastexp bit-trick, split in halves: DVE makes
                            # bf16-bits ints from psum; Pool masks half 0,
                            # DVE masks half 1 (bounded latency for the AV)
                            e16 = e16_pool.tile(
                                [128, 1024], I16, tag="e16", name="e16"
                            )
                            for hh in range(2):
                                csl = slice(hh * 512, (hh + 1) * 512)
                                nc.vector.tensor_scalar(
                                    e16[:, csl], ps[:, csl], FE_C1, FE_C2,
                                    op0=ALU.mult, op1=ALU.add,
                                )
                                mop = nc.gpsimd if hh == 0 else nc.vector
                                mop.tensor_tensor(
                                    e[:, csl],
                                    e16[:, csl].bitcast(BF16),
                                    mk_sb[:, tb, sp * 1024 + hh * 512 :
                                          sp * 1024 + (hh + 1) * 512],
                                    op=ALU.mult,
                                )
                        else:
                            ex = ex_pool.tile(
                                [128, 1024], BF16, tag="ex", name="ex"
                            )
                            nc.scalar.activation(ex[:], ps[:], AF.Exp)
                            nc.vector.tensor_tensor(
                                e[:], ex[:], mk_sb[:, tb, psl], op=ALU.mult
                            )

                    def emit_av(tb, py0, py1, first, last):
                        e = e_tiles.pop(tb)
                        nc.tensor.matmul(
                            py0[:],
                            v2[:, m, tb, half, :],
                            e[:, 0:512],
                            start=first,
                            stop=last,
                        )
                        nc.tensor.matmul(
                            py1[:],
                            v2[:, m, tb, half, :],
                            e[:, 512:1024],
                            start=first,
                            stop=last,
                        )

                    psl = slice(sp * 1024, (sp + 1) * 1024)
                    py0 = y_ps.tile([65, 512], F32, tag="y", name="py0")
                    py1 = y_ps.tile([65, 512], F32, tag="y", name="py1")
                    vp = (m, half, sp) == (0, 0, 0)  # inline vproj this pass
                    for gs in range(0, NT, 4):
                        gi = gs // 4
                        ft = gs + FAST_TB if FAST_TB is not None else -1
                        for fn, args in interleave.get((key, gi, 0), ()):
                            fn(*args)
                        if ft >= 0:
                            if vp:
                                emit_vproj(ft)
                            emit_scores_e(ft, True)
                        acts = [t for t in range(gs, gs + 4) if t != ft]
                        for i, t in enumerate(acts):
                            if vp:
                                emit_vproj(t)
                            emit_scores_e(t, False)
                            emit_av(
                                t, py0, py1,
                                gs == 0 and i == 0,
                                ft < 0 and gs == NT - 4 and i == len(acts) - 1,
                            )
                            if i == 1:
                                for fn, args in interleave.get(
                                    (key, gi, 1), ()
                                ):
                                    fn(*args)
                        if ft >= 0:
                            emit_av(ft, py0, py1, False, gs == NT - 4)
                    for hh, py in ((0, py0), (1, py1)):
                        osl = slice(
                            sp * 1024 + hh * 512, sp * 1024 + (hh + 1) * 512
                        )
                        y_sb = y_pool.tile([65, 512], F32, tag="y_sb")
                        nc.vector.tensor_copy(y_sb[:], py[:])
                        nc.sync.dma_start(yt_d[j, :, osl], y_sb[:])

    nc.compile()
    return nc


_NC_CACHE = {}


def _get_nc():
    if "nc" not in _NC_CACHE:
        _NC_CACHE["nc"] = build_nc()
    return _NC_CACHE["nc"]


def make_in_maps(X, A, W, b):
    X = np.ascontiguousarray(np.asarray(X), dtype=np.float32)
    A = np.asarray(A)
    W = np.ascontiguousarray(np.asarray(W), dtype=np.float32)
    b = np.ascontiguousarray(np.asarray(b), dtype=np.float32)
    scale = np.float32(1.0 / np.sqrt(np.float32(H)))
    d = np.arange(D)

    xts = [np.ascontiguousarray(X[bb].T).astype(np.float16) for bb in range(B)]
    # multiplicative mask, transposed to [t, s], bf16 (exactly 0.0 / 1.0)
    msks = [
        np.ascontiguousarray(A[bb].T).astype(ml_dtypes.bfloat16) for bb in range(B)
    ]

    # per head-group weight/bias packs (head-pair layout)
    packs = []
    for hg in range(2):
        wqk = np.empty((F, NPAIR * 256), np.float32)
        wv = np.empty((F, NPAIR * 128), np.float32)
        bias = np.empty((128, 2 * NPAIR), np.float32)
        vbias = np.empty((128, NPAIR * 128), np.float32)
        for m in range(NPAIR):
            for half in range(2):
                h = hg * HPC + 2 * m + half
                qc = d * 24 + h
                kc = d * 24 + 8 + h
                vc = d * 24 + 16 + h
                c0 = m * 256 + half * 64
                wqk[:, c0 : c0 + 64] = W[:, qc] * scale
                wqk[:, c0 + 128 : c0 + 192] = W[:, kc]
                wv[:, m * 128 + half * 64 : m * 128 + (half + 1) * 64] = W[:, vc]
                rsl = slice(64 * half, 64 * (half + 1))
                bias[rsl, 2 * m] = b[qc] * scale
                bias[rsl, 2 * m + 1] = b[kc]
                vbias[:, m * 128 + half * 64 : m * 128 + (half + 1) * 64] = b[vc][
                    None, :
                ]
        packs.append(
            (wqk.astype(np.float16), wv.astype(np.float16), bias, vbias)
        )

    in_maps = []
    for c in range(N_CORES):
        bb = c % B
        hg = c // B
        wqk, wv, bias, vbias = packs[hg]
        in_maps.append(
            {
                "xt": xts[bb],
                "msk": msks[bb],
                "wqk": wqk,
                "wv": wv,
                "bias": bias,
                "vbias": vbias,
            }
        )
    return in_maps


def assemble_output(results):
    Y = np.empty((B, S, D * H), np.float32)
    Yv = Y.reshape(B, S, D, H)
    for c in range(N_CORES):
        bb = c % B
        hg = c // B
        yt = results[c]["yt"]  # [HPC, 65, S]
        for j in range(HPC):
            h = hg * HPC + j
            Yv[bb, :, :, h] = (yt[j, 0:64, :] / yt[j, 64:65, :]).T
    return Y


def kernel(X, A, W, b):
    nc = _get_nc()
    in_maps = make_in_maps(X, A, W, b)
    res = bass_utils.run_bass_kernel_spmd(
        nc, in_maps, core_ids=list(range(N_CORES))
    ).results
    return assemble_output(res)
